# revision 10
# baseline (speedup 1.0000x reference)
"""Trainium2 Bass kernel for nn_Attention (B=4, N=2048, D=1024, H=16, DH=64).

Sharding: 8 cores = 4 batches x 2 head-halves (tensor-parallel heads).
Per core: qT/kT/v projections (bf16 matmuls, fp32 psum) -> RoPE (PE
permutation matmul + DVE combine, written directly to qTr/kTr) -> scores^T
per jt-pair into a [128,2,512] psum -> ONE 1024-wide exp per (head,
jt-pair) -> P^T @ [v|1] per jt (fused row-sum via ones column) -> deferred
normalization (reciprocal + fp32 DMA partition-broadcast + DVE multiply)
-> output projection partial (Wout row-shard).
Host: out[b] = partial[2b] + partial[2b+1] + bout.

Scheduling: the PE executes in emission order, so the attention inner loop
(ACT-bound: 2.08us exp vs 1.71us matmul per group) is interleaved with
micro-fillers -- next pair's projection/rope matmuls, v-projection (pair 0),
and output-projection chunks (pair 3) -- emitted at points inside each
group where PE would otherwise stall on the exp->PV dependency.

Engine budget per core per pass: PE ~765k cyc ~319us (qkv 196k, S^T 262k,
PV 262k, outproj 65k, rope 16k); ACT 256 exps ~266us; DVE ~145us;
Pool (psum evictions) ~55us; DMA ~45us.
"""

import sys

for _p in ("/opt/trn_rl_repo",):
    if _p not in sys.path:
        sys.path.insert(0, _p)

import numpy as np
import ml_dtypes

import concourse.bass as bass
import concourse.tile as tile
from concourse import library_config, mybir
from concourse.bass_utils import run_bass_kernel_spmd

BF16 = mybir.dt.bfloat16
F32 = mybir.dt.float32
NP_BF16 = ml_dtypes.bfloat16
EXP = mybir.ActivationFunctionType.Exp
IDENT = mybir.ActivationFunctionType.Identity

P = 128          # partitions
D = 1024         # model dim
INNER = 512      # per-core inner dim (8 heads * 64)
NH = 8           # heads per core
DH = 64          # head dim
KT = D // P      # 8 contraction tiles for projections
SCALE = DH ** -0.5


def build_nc(N=2048, n_cores=8, split_waits=True, repeat=1):
    """Build the per-core bass program (SPMD: same program, different data)."""
    NJT = N // P           # 16 j tiles
    NA = NJT // 2          # 8 jt-pairs
    IC = 512               # i-chunk
    NIC = N // IC          # 4
    PCH = 512              # projection col chunk
    NCX = N // PCH         # 4

    nc = bass.Bass("TRN2", target_bir_lowering=False, debug=False,
                   num_devices=n_cores)

    xT = nc.dram_tensor("xT", [D, N], BF16, kind="ExternalInput").ap()
    wq = nc.dram_tensor("wq", [D, INNER], BF16, kind="ExternalInput").ap()
    wk = nc.dram_tensor("wk", [D, INNER], BF16, kind="ExternalInput").ap()
    wv = nc.dram_tensor("wv", [D, INNER], BF16, kind="ExternalInput").ap()
    wout = nc.dram_tensor("wout", [INNER, D], BF16, kind="ExternalInput").ap()
    cosT = nc.dram_tensor("cosT", [P, N], BF16, kind="ExternalInput").ap()
    sinT = nc.dram_tensor("sinT", [P, N], BF16, kind="ExternalInput").ap()
    rT = nc.dram_tensor("rT", [P, P], BF16, kind="ExternalInput").ap()
    out = nc.dram_tensor("out", [N, D], F32, kind="ExternalOutput").ap()

    with tile.TileContext(nc) as tc:
        with tc.tile_pool(name="persist", bufs=1) as pp:
            wq_sb = pp.tile([P, KT, INNER], BF16, tag="wq")
            wk_sb = pp.tile([P, KT, INNER], BF16, tag="wk")
            wv_sb = pp.tile([P, KT, INNER], BF16, tag="wv")
            wout_sb = pp.tile([P, 4, D], BF16, tag="wout")
            cos_sb = pp.tile([P, N], BF16, tag="cos")
            sin_sb = pp.tile([P, N], BF16, tag="sin")
            rT_sb = pp.tile([P, P], BF16, tag="rT")
            xt_all = pp.tile([P, KT, N], BF16, tag="xt")
            qT = pp.tile([P, 4, N], BF16, tag="qT")
            kT = pp.tile([P, 4, N], BF16, tag="kT")
            qTr = pp.tile([P, 4, N], BF16, tag="qTr")
            kTr = pp.tile([P, 4, N], BF16, tag="kTr")
            vA = pp.tile([P, NJT, NH, DH + 1], BF16, tag="vA")
            outT = pp.tile([P, 4, N], BF16, tag="outT")

            for kt in range(KT):
                nc.sync.dma_start(out=wq_sb[:, kt, :],
                                  in_=wq.rearrange("(kt p) i -> p kt i", p=P)[:, kt, :])
                nc.sync.dma_start(out=wk_sb[:, kt, :],
                                  in_=wk.rearrange("(kt p) i -> p kt i", p=P)[:, kt, :])
                nc.sync.dma_start(out=wv_sb[:, kt, :],
                                  in_=wv.rearrange("(kt p) i -> p kt i", p=P)[:, kt, :])
            for p4 in range(4):
                nc.sync.dma_start(out=wout_sb[:, p4, :],
                                  in_=wout.rearrange("(pt p) d -> p pt d", p=P)[:, p4, :])
            nc.sync.dma_start(out=cos_sb, in_=cosT)
            nc.sync.dma_start(out=sin_sb, in_=sinT)
            nc.sync.dma_start(out=rT_sb, in_=rT)
            # ones column for fused row-sum in PV matmul (on ACT so the PV
            # matmul's waits stay within the MM sync-command limit)
            nc.scalar.activation(
                vA[:, :, :, DH:DH + 1],
                cos_sb[:, 0:NJT * NH].rearrange("p (a b) -> p a b", a=NJT)[:, :, :, None],
                IDENT, bias=1.0, scale=0.0)

            for _rep in range(repeat):
                with tc.tile_pool(name="pt_pool", bufs=2) as ptp, \
                     tc.tile_pool(name="nrm", bufs=1) as nrm, \
                     tc.tile_pool(name="rope_t", bufs=2) as rp, \
                     tc.tile_pool(name="fin_p", bufs=2) as fin_p, \
                     tc.tile_pool(name="ps", bufs=1, space="PSUM") as ps:
                    xT_r = xT.rearrange("(kt p) n -> p kt n", p=P)
                    for xc in range(NCX):
                        xsl = slice(xc * PCH, (xc + 1) * PCH)
                        for kt in range(KT):
                            nc.sync.dma_start(out=xt_all[:, kt, xsl],
                                              in_=xT_r[:, kt, xsl])

                    _ctr = {"n": 0}

                    def aux():
                        _ctr["n"] += 1
                        return ps.tile([P, 2, 512], F32, tag="ps_aux", bufs=1,
                                       name=f"aux{_ctr['n']}")

                    # ---- filler micro-units (closures, ~1 PE matmul each) ----
                    def proj_closures(pair, c):
                        """q+k projection for col chunk c of `pair`."""
                        csl = slice(c * PCH, (c + 1) * PCH)
                        msl = slice(pair * P, (pair + 1) * P)
                        cl = []
                        box = {}

                        def mk(kt, half, w_sb):
                            def f():
                                if "t" not in box:
                                    box["t"] = aux()
                                nc.tensor.matmul(box["t"][:, half, :],
                                                 w_sb[:, kt, msl],
                                                 xt_all[:, kt, csl],
                                                 start=(kt == 0), stop=(kt == KT - 1))
                            return f

                        for kt in range(KT):
                            cl.append(mk(kt, 0, wq_sb))
                        cl.append(lambda: nc.vector.tensor_copy(
                            qT[:, pair, csl], box["t"][:, 0, :]))
                        for kt in range(KT):
                            cl.append(mk(kt, 1, wk_sb))
                        cl.append(lambda: nc.vector.tensor_copy(
                            kT[:, pair, csl], box["t"][:, 1, :]))
                        return cl

                    def rope_closures(pair, c, is_k):
                        csl = slice(c * PCH, (c + 1) * PCH)
                        src, dst = (kT, kTr) if is_k else (qT, qTr)
                        half = 1 if is_k else 0
                        box = {}

                        def rot():
                            # rotate_half = partition permutation: 4 SBUF->SBUF
                            # DMAs (DVE hwdge queue, off the PE); the sign is
                            # folded into the host-precomputed signed sinT.
                            rb = rp.tile([P, PCH], BF16, tag="rot",
                                         name=f"rb{pair}_{c}_{half}")
                            for h2 in range(2):
                                b = h2 * 64
                                nc.sync.dma_start(
                                    out=rb[b:b + 32, :],
                                    in_=src[b + 32:b + 64, pair, csl])
                                nc.sync.dma_start(
                                    out=rb[b + 32:b + 64, :],
                                    in_=src[b:b + 32, pair, csl])
                            box["rb"] = rb

                        def comb():
                            t1 = rp.tile([P, PCH], BF16, tag="t1")
                            nc.vector.tensor_mul(t1, src[:, pair, csl],
                                                 cos_sb[:, csl])
                            t2 = rp.tile([P, PCH], BF16, tag="t2")
                            nc.vector.tensor_mul(t2, box["rb"],
                                                 sin_sb[:, csl])
                            nc.vector.tensor_add(dst[:, pair, csl], t1, t2)

                        return [rot, comb]

                    def emit_v(a):
                        """v projection for j-tiles 2a, 2a+1 (one psum fill)."""
                        pv = aux()
                        for s in range(2):
                            jt = 2 * a + s
                            r2sl = slice(jt * P, (jt + 1) * P)
                            for kt in range(KT):
                                nc.tensor.matmul(pv[:, s, :],
                                                 xt_all[:, kt, r2sl],
                                                 wv_sb[:, kt, :],
                                                 start=(kt == 0), stop=(kt == KT - 1))
                        nc.vector.tensor_copy(
                            vA[:, 2 * a:2 * a + 2, :, 0:DH],
                            pv.rearrange("p s (h d) -> p s h d", h=NH))

                    def out_closures(rtile):
                        """output projection for row tile rtile (2 col halves,
                        one psum bank each)."""
                        rsl = slice(rtile * P, (rtile + 1) * P)
                        cl = []
                        box = {}

                        def mk(p4, half):
                            def f():
                                if "t" not in box:
                                    box["t"] = aux()
                                nsl = slice(half * 512, (half + 1) * 512)
                                nc.tensor.matmul(
                                    box["t"][:, half, :],
                                    outT[:, p4, rsl], wout_sb[:, p4, nsl],
                                    start=(p4 == 0), stop=(p4 == 3))
                            return f

                        for half in range(2):
                            for p4 in range(4):
                                cl.append(mk(p4, half))

                        def fin():
                            ft = fin_p.tile([P, D], F32, tag="fin",
                                            name=f"fin{rtile}")
                            nc.vector.tensor_copy(
                                ft, box["t"].rearrange("p a b -> p (a b)"))
                            nc.sync.dma_start(out=out[rsl, :], in_=ft)
                        cl.append(fin)
                        return cl

                    # ---- attention pieces ----
                    def scores_exp(pair, ic, a, lc, pts):
                        isl = slice(ic * IC, (ic + 1) * IC)
                        prow = slice(lc * DH, (lc + 1) * DH)
                        pssh = ps.tile([P, 2, IC], F32, tag=f"ps_s{lc}", bufs=1,
                                       name=f"pssh{pair}_{ic}_{a}_{lc}")
                        for s in range(2):
                            jt = 2 * a + s
                            jsl = slice(jt * P, (jt + 1) * P)
                            nc.tensor.matmul(pssh[:, s, :],
                                             kTr[prow, pair, jsl],
                                             qTr[prow, pair, isl],
                                             start=True, stop=True)
                        pt = ptp.tile([P, 2, IC], BF16, tag=f"pt{lc}",
                                      name=f"pt{pair}_{ic}_{a}_{lc}")
                        nc.scalar.activation(pt, pssh, EXP, scale=SCALE)
                        pts[lc] = pt

                    def pv(pair, ic, a, lc, ps_o, pts):
                        h = pair * 2 + lc
                        for s in range(2):
                            nc.tensor.matmul(ps_o[lc][0:DH + 1, :],
                                             vA[:, 2 * a + s, h, :],
                                             pts[lc][:, s, :],
                                             start=(a == 0 and s == 0),
                                             stop=(a == NA - 1 and s == 1))

                    def norm(pair, ic, ps_o):
                        isl = slice(ic * IC, (ic + 1) * IC)
                        for lc in range(2):
                            prow = slice(lc * DH, (lc + 1) * DH)
                            oc = nrm.tile([DH + 1, IC], F32, tag=f"oc{lc}",
                                          name=f"oc{pair}_{ic}_{lc}")
                            nc.vector.tensor_copy(oc, ps_o[lc][0:DH + 1, :])
                            lrec = nrm.tile([1, IC], F32, tag=f"lrec{lc}",
                                            name=f"lrec{pair}_{ic}_{lc}")
                            nc.vector.reciprocal(lrec, oc[DH:DH + 1, :])
                            lb = nrm.tile([DH, IC], F32, tag=f"lb{lc}",
                                          name=f"lb{pair}_{ic}_{lc}")
                            nc.sync.dma_start(
                                out=lb,
                                in_=lrec[:, None, :].to_broadcast((1, DH, IC)))
                            nc.vector.tensor_mul(outT[prow, pair, isl],
                                                 oc[0:DH, :], lb)

                    # ---- warmup: pair-0 proj+rope, first v pairs ----
                    for c in range(NCX):
                        for f in proj_closures(0, c):
                            f()
                    for c in range(NCX):
                        for f in rope_closures(0, c, False):
                            f()
                        for f in rope_closures(0, c, True):
                            f()
                    emit_v(0)
                    emit_v(1)

                    # ---- main: attention with interleaved fillers ----
                    for pair in range(4):
                        fq = []
                        if pair < 3:
                            for c in range(NCX):
                                fq += proj_closures(pair + 1, c)
                                fq += rope_closures(pair + 1, c, False)
                                fq += rope_closures(pair + 1, c, True)
                        state = {"points": 2 * NA * NIC}

                        def drain():
                            k = -(-len(fq) // max(1, state["points"]))
                            state["points"] -= 1
                            while k > 0 and fq:
                                fq.pop(0)()
                                k -= 1

                        for ic in range(NIC):
                            ps_o = [ps.tile([P, IC], F32, tag="ps_o0", bufs=1,
                                            name=f"ps_o0_{pair}_{ic}"),
                                    ps.tile([P, IC], F32, tag="ps_o1", bufs=1,
                                            name=f"ps_o1_{pair}_{ic}")]
                            for a in range(NA):
                                pts = {}
                                scores_exp(pair, ic, a, 0, pts)
                                scores_exp(pair, ic, a, 1, pts)
                                drain()
                                pv(pair, ic, a, 0, ps_o, pts)
                                pv(pair, ic, a, 1, ps_o, pts)
                                drain()
                                if pair == 0 and ic == 0 and a + 2 < NA:
                                    emit_v(a + 2)
                            norm(pair, ic, ps_o)
                            if pair == 3:
                                for rt in range(4 * ic, 4 * ic + 4):
                                    fq += out_closures(rt)
                        for f in fq:
                            f()

    if split_waits:
        _split_sync_waits(nc)
    return nc


_SYNC_EXEMPT = {"InstEventSemaphore", "InstAllEngineBarrier",
                "InstNoOp", "InstUnconditionalBranch", "InstCompareAndBranch",
                "InstHalt", "InstBranchHint"}


def _split_sync_waits(nc, cap_total=2):
    """Compact TPB instruction encodings only fit ~2 sync commands
    (waits+updates).  Tile can park many waits on one instruction; hoist the
    excess onto same-engine InstNoOps inserted immediately before (waits
    strictly earlier in the same stream — always safe)."""
    for fn in nc.m.functions:
        for bb in fn.blocks:
            il = bb.instructions
            n = 0
            while n < len(il):
                i = il[n]
                nm = type(i).__name__
                si = i.sync_info
                if nm in _SYNC_EXEMPT or si is None:
                    n += 1
                    continue
                waits = list(si.on_wait or [])
                upds = list(si.on_update or [])
                allowed = 0 if len(upds) >= 2 else 1
                if len(waits) <= allowed:
                    n += 1
                    continue
                keep = waits[-allowed:] if allowed else []
                excess = waits[:len(waits) - allowed]
                pos = n
                while excess:
                    chunk, excess = excess[:1], excess[1:]
                    nop = mybir.InstNoOp(
                        name=nc.get_next_instruction_name(),
                        engine=i.engine,
                        bass_nofuse=True,
                        sync_info=mybir.SyncInfo(on_wait=chunk, on_update=[]),
                    )
                    il.insert(pos, nop)
                    pos += 1
                si.on_wait = keep
                n = pos + 1


def _rot_matrix_T():
    """R^T such that (R @ tT) = rotate_half(t)^T in [h*64+d, n] layout."""
    r64 = np.zeros((DH, DH), dtype=np.float32)
    for dp in range(32):
        r64[dp, dp + 32] = -1.0
        r64[dp + 32, dp] = 1.0
    r = np.zeros((P, P), dtype=np.float32)
    r[:DH, :DH] = r64
    r[DH:, DH:] = r64
    return np.ascontiguousarray(r.T.astype(NP_BF16))


def make_in_maps(x, rotary_emb, Wq, Wkv, Wout, n_cores=8):
    B, N, Dm = x.shape
    rT = _rot_matrix_T()
    cosT = np.tile(np.cos(rotary_emb.astype(np.float64)).T, (2, 1)).astype(NP_BF16)
    sin_f = np.sin(rotary_emb.astype(np.float64)).T  # [64, N]
    sin_f[:32, :] *= -1.0  # sign of rotate_half folded into sin
    sinT = np.tile(sin_f, (2, 1)).astype(NP_BF16)
    cosT = np.ascontiguousarray(cosT)
    sinT = np.ascontiguousarray(sinT)
    wk_full = Wkv[:, :Dm]
    wv_full = Wkv[:, Dm:]
    in_maps = []
    for c in range(n_cores):
        b, hh = c // 2, c % 2
        sl = slice(hh * INNER, (hh + 1) * INNER)
        in_maps.append({
            "xT": np.ascontiguousarray(x[b].T).astype(NP_BF16),
            "wq": np.ascontiguousarray(Wq[:, sl]).astype(NP_BF16),
            "wk": np.ascontiguousarray(wk_full[:, sl]).astype(NP_BF16),
            "wv": np.ascontiguousarray(wv_full[:, sl]).astype(NP_BF16),
            "wout": np.ascontiguousarray(Wout[sl, :]).astype(NP_BF16),
            "cosT": cosT,
            "sinT": sinT,
            "rT": rT,
        })
    return in_maps


_NC_CACHE = {}


def kernel(x, rotary_emb, Wq, Wkv, Wout, bout, _trace=False):
    B, N, Dm = x.shape
    if "nc" not in _NC_CACHE:
        _NC_CACHE["nc"] = build_nc(N=N)
    nc = _NC_CACHE["nc"]
    in_maps = make_in_maps(x, rotary_emb, Wq, Wkv, Wout)
    res = run_bass_kernel_spmd(nc, in_maps, core_ids=list(range(8)),
                               trace=_trace)
    outs = [res.results[c]["out"] for c in range(8)]
    full = np.empty((B, N, Dm), dtype=np.float32)
    for b in range(B):
        full[b] = outs[2 * b] + outs[2 * b + 1] + bout[None, :].astype(np.float32)
    if _trace:
        return full, res
    return full


# revision 11
# speedup vs baseline: 2.0018x; 2.0018x over previous
"""Trainium2 Bass kernel for nn_Attention (B=4, N=2048, D=1024, H=16, DH=64).

Sharding: 8 cores = 4 batches x 2 head-halves (tensor-parallel heads).
Per core: qT/kT/v projections (bf16 matmuls, fp32 psum) -> RoPE (PE
permutation matmul + DVE combine, written directly to qTr/kTr) -> scores^T
per jt-pair into a [128,2,512] psum -> ONE 1024-wide exp per (head,
jt-pair) -> P^T @ [v|1] per jt (fused row-sum via ones column) -> deferred
normalization (reciprocal + fp32 DMA partition-broadcast + DVE multiply)
-> output projection partial (Wout row-shard).
Host: out[b] = partial[2b] + partial[2b+1] + bout.

Scheduling: the PE executes in emission order, so the attention inner loop
(ACT-bound: 2.08us exp vs 1.71us matmul per group) is interleaved with
micro-fillers -- next pair's projection/rope matmuls, v-projection (pair 0),
and output-projection chunks (pair 3) -- emitted at points inside each
group where PE would otherwise stall on the exp->PV dependency.

Engine budget per core per pass: PE ~765k cyc ~319us (qkv 196k, S^T 262k,
PV 262k, outproj 65k, rope 16k); ACT 256 exps ~266us; DVE ~145us;
Pool (psum evictions) ~55us; DMA ~45us.
"""

import sys

for _p in ("/opt/trn_rl_repo",):
    if _p not in sys.path:
        sys.path.insert(0, _p)

import numpy as np
import ml_dtypes

import concourse.bass as bass
import concourse.tile as tile
from concourse import library_config, mybir
from concourse.bass_utils import run_bass_kernel_spmd

BF16 = mybir.dt.bfloat16
F32 = mybir.dt.float32
NP_BF16 = ml_dtypes.bfloat16
EXP = mybir.ActivationFunctionType.Exp
IDENT = mybir.ActivationFunctionType.Identity

P = 128          # partitions
D = 1024         # model dim
INNER = 512      # per-core inner dim (8 heads * 64)
NH = 8           # heads per core
DH = 64          # head dim
KT = D // P      # 8 contraction tiles for projections
SCALE = DH ** -0.5


def build_nc(N=2048, n_cores=8, split_waits=True, repeat=1):
    """Build the per-core bass program (SPMD: same program, different data)."""
    NJT = N // P           # 16 j tiles
    NA = NJT // 2          # 8 jt-pairs
    IC = 512               # i-chunk
    NIC = N // IC          # 4
    PCH = 512              # projection col chunk
    NCX = N // PCH         # 4

    nc = bass.Bass("TRN2", target_bir_lowering=False, debug=False,
                   num_devices=n_cores)

    xT = nc.dram_tensor("xT", [D, N], BF16, kind="ExternalInput").ap()
    wq = nc.dram_tensor("wq", [D, INNER], BF16, kind="ExternalInput").ap()
    wk = nc.dram_tensor("wk", [D, INNER], BF16, kind="ExternalInput").ap()
    wv = nc.dram_tensor("wv", [D, INNER], BF16, kind="ExternalInput").ap()
    wout = nc.dram_tensor("wout", [INNER, D], BF16, kind="ExternalInput").ap()
    cosT = nc.dram_tensor("cosT", [P, N], BF16, kind="ExternalInput").ap()
    sinT = nc.dram_tensor("sinT", [P, N], BF16, kind="ExternalInput").ap()
    rT = nc.dram_tensor("rT", [P, P], BF16, kind="ExternalInput").ap()
    out = nc.dram_tensor("out", [N, D], F32, kind="ExternalOutput").ap()

    with tile.TileContext(nc) as tc:
        with tc.tile_pool(name="persist", bufs=1) as pp:
            wq_sb = pp.tile([P, KT, INNER], BF16, tag="wq")
            wk_sb = pp.tile([P, KT, INNER], BF16, tag="wk")
            wv_sb = pp.tile([P, KT, INNER], BF16, tag="wv")
            wout_sb = pp.tile([P, 4, D], BF16, tag="wout")
            cos_sb = pp.tile([P, N], BF16, tag="cos")
            sin_sb = pp.tile([P, N], BF16, tag="sin")
            rT_sb = pp.tile([P, P], BF16, tag="rT")
            xt_all = pp.tile([P, KT, N], BF16, tag="xt")
            qT = pp.tile([P, 4, N], BF16, tag="qT")
            kT = pp.tile([P, 4, N], BF16, tag="kT")
            qTr = pp.tile([P, 4, N], BF16, tag="qTr")
            kTr = pp.tile([P, 4, N], BF16, tag="kTr")
            vA = pp.tile([P, NJT, NH, DH + 1], BF16, tag="vA")
            outT = pp.tile([P, 4, N], BF16, tag="outT")

            for kt in range(KT):
                nc.sync.dma_start(out=wq_sb[:, kt, :],
                                  in_=wq.rearrange("(kt p) i -> p kt i", p=P)[:, kt, :])
                nc.sync.dma_start(out=wk_sb[:, kt, :],
                                  in_=wk.rearrange("(kt p) i -> p kt i", p=P)[:, kt, :])
                nc.sync.dma_start(out=wv_sb[:, kt, :],
                                  in_=wv.rearrange("(kt p) i -> p kt i", p=P)[:, kt, :])
            for p4 in range(4):
                nc.sync.dma_start(out=wout_sb[:, p4, :],
                                  in_=wout.rearrange("(pt p) d -> p pt d", p=P)[:, p4, :])
            nc.sync.dma_start(out=cos_sb, in_=cosT)
            nc.sync.dma_start(out=sin_sb, in_=sinT)
            nc.sync.dma_start(out=rT_sb, in_=rT)
            # ones column for fused row-sum in PV matmul (on ACT so the PV
            # matmul's waits stay within the MM sync-command limit)
            nc.scalar.activation(
                vA[:, :, :, DH:DH + 1],
                cos_sb[:, 0:NJT * NH].rearrange("p (a b) -> p a b", a=NJT)[:, :, :, None],
                IDENT, bias=1.0, scale=0.0)

            for _rep in range(repeat):
                with tc.tile_pool(name="pt_pool", bufs=2) as ptp, \
                     tc.tile_pool(name="nrm", bufs=1) as nrm, \
                     tc.tile_pool(name="rope_t", bufs=2) as rp, \
                     tc.tile_pool(name="fin_p", bufs=2) as fin_p, \
                     tc.tile_pool(name="ps", bufs=1, space="PSUM") as ps:
                    xT_r = xT.rearrange("(kt p) n -> p kt n", p=P)
                    for xc in range(NCX):
                        xsl = slice(xc * PCH, (xc + 1) * PCH)
                        for kt in range(KT):
                            nc.sync.dma_start(out=xt_all[:, kt, xsl],
                                              in_=xT_r[:, kt, xsl])

                    _ctr = {"n": 0}

                    def aux():
                        _ctr["n"] += 1
                        return ps.tile([P, 2, 512], F32, tag="ps_aux", bufs=1,
                                       name=f"aux{_ctr['n']}")

                    # ---- filler micro-units (closures, ~1 PE matmul each) ----
                    def proj_closures(pair, c):
                        """q+k projection for col chunk c of `pair`."""
                        csl = slice(c * PCH, (c + 1) * PCH)
                        msl = slice(pair * P, (pair + 1) * P)
                        cl = []
                        box = {}

                        def mk(kt, half, w_sb):
                            def f():
                                if "t" not in box:
                                    box["t"] = aux()
                                nc.tensor.matmul(box["t"][:, half, :],
                                                 w_sb[:, kt, msl],
                                                 xt_all[:, kt, csl],
                                                 start=(kt == 0), stop=(kt == KT - 1))
                            return f

                        for kt in range(KT):
                            cl.append(mk(kt, 0, wq_sb))
                        cl.append(lambda: nc.vector.tensor_copy(
                            qT[:, pair, csl], box["t"][:, 0, :]))
                        for kt in range(KT):
                            cl.append(mk(kt, 1, wk_sb))
                        cl.append(lambda: nc.vector.tensor_copy(
                            kT[:, pair, csl], box["t"][:, 1, :]))
                        return cl

                    def rope_closures(pair, c, is_k):
                        csl = slice(c * PCH, (c + 1) * PCH)
                        src, dst = (kT, kTr) if is_k else (qT, qTr)
                        half = 1 if is_k else 0
                        box = {}

                        def rot():
                            box["t"] = aux()
                            nc.tensor.matmul(box["t"][:, half, :], rT_sb,
                                             src[:, pair, csl],
                                             start=True, stop=True)

                        def comb():
                            t1 = rp.tile([P, PCH], BF16, tag="t1")
                            nc.vector.tensor_mul(t1, src[:, pair, csl],
                                                 cos_sb[:, csl])
                            t2 = rp.tile([P, PCH], BF16, tag="t2")
                            nc.vector.tensor_mul(t2, box["t"][:, half, :],
                                                 sin_sb[:, csl])
                            nc.vector.tensor_add(dst[:, pair, csl], t1, t2)

                        return [rot, comb]

                    def emit_v(a):
                        """v projection for j-tiles 2a, 2a+1 (one psum fill)."""
                        pv = aux()
                        for s in range(2):
                            jt = 2 * a + s
                            r2sl = slice(jt * P, (jt + 1) * P)
                            for kt in range(KT):
                                nc.tensor.matmul(pv[:, s, :],
                                                 xt_all[:, kt, r2sl],
                                                 wv_sb[:, kt, :],
                                                 start=(kt == 0), stop=(kt == KT - 1))
                        nc.vector.tensor_copy(
                            vA[:, 2 * a:2 * a + 2, :, 0:DH],
                            pv.rearrange("p s (h d) -> p s h d", h=NH))

                    def out_closures(rtile):
                        """output projection for row tile rtile (2 col halves,
                        one psum bank each)."""
                        rsl = slice(rtile * P, (rtile + 1) * P)
                        cl = []
                        box = {}

                        def mk(p4, half):
                            def f():
                                if "t" not in box:
                                    box["t"] = aux()
                                nsl = slice(half * 512, (half + 1) * 512)
                                nc.tensor.matmul(
                                    box["t"][:, half, :],
                                    outT[:, p4, rsl], wout_sb[:, p4, nsl],
                                    start=(p4 == 0), stop=(p4 == 3))
                            return f

                        for half in range(2):
                            for p4 in range(4):
                                cl.append(mk(p4, half))

                        def fin():
                            ft = fin_p.tile([P, D], F32, tag="fin",
                                            name=f"fin{rtile}")
                            nc.vector.tensor_copy(
                                ft, box["t"].rearrange("p a b -> p (a b)"))
                            nc.sync.dma_start(out=out[rsl, :], in_=ft)
                        cl.append(fin)
                        return cl

                    # ---- attention pieces ----
                    def scores_exp(pair, ic, a, lc, pts):
                        isl = slice(ic * IC, (ic + 1) * IC)
                        prow = slice(lc * DH, (lc + 1) * DH)
                        pssh = ps.tile([P, 2, IC], F32, tag=f"ps_s{lc}", bufs=1,
                                       name=f"pssh{pair}_{ic}_{a}_{lc}")
                        for s in range(2):
                            jt = 2 * a + s
                            jsl = slice(jt * P, (jt + 1) * P)
                            nc.tensor.matmul(pssh[:, s, :],
                                             kTr[prow, pair, jsl],
                                             qTr[prow, pair, isl],
                                             start=True, stop=True)
                        pt = ptp.tile([P, 2, IC], BF16, tag=f"pt{lc}",
                                      name=f"pt{pair}_{ic}_{a}_{lc}")
                        nc.scalar.activation(pt, pssh, EXP, scale=SCALE)
                        pts[lc] = pt

                    def pv(pair, ic, a, lc, ps_o, pts):
                        h = pair * 2 + lc
                        for s in range(2):
                            nc.tensor.matmul(ps_o[lc][0:DH + 1, :],
                                             vA[:, 2 * a + s, h, :],
                                             pts[lc][:, s, :],
                                             start=(a == 0 and s == 0),
                                             stop=(a == NA - 1 and s == 1))

                    def norm(pair, ic, ps_o):
                        isl = slice(ic * IC, (ic + 1) * IC)
                        for lc in range(2):
                            prow = slice(lc * DH, (lc + 1) * DH)
                            oc = nrm.tile([DH + 1, IC], F32, tag=f"oc{lc}",
                                          name=f"oc{pair}_{ic}_{lc}")
                            nc.vector.tensor_copy(oc, ps_o[lc][0:DH + 1, :])
                            lrec = nrm.tile([1, IC], F32, tag=f"lrec{lc}",
                                            name=f"lrec{pair}_{ic}_{lc}")
                            nc.vector.reciprocal(lrec, oc[DH:DH + 1, :])
                            lb = nrm.tile([DH, IC], F32, tag=f"lb{lc}",
                                          name=f"lb{pair}_{ic}_{lc}")
                            nc.sync.dma_start(
                                out=lb,
                                in_=lrec[:, None, :].to_broadcast((1, DH, IC)))
                            nc.vector.tensor_mul(outT[prow, pair, isl],
                                                 oc[0:DH, :], lb)

                    # ---- warmup: pair-0 proj+rope, first v pairs ----
                    for c in range(NCX):
                        for f in proj_closures(0, c):
                            f()
                    for c in range(NCX):
                        for f in rope_closures(0, c, False):
                            f()
                        for f in rope_closures(0, c, True):
                            f()
                    emit_v(0)
                    emit_v(1)

                    # ---- main: attention with interleaved fillers ----
                    for pair in range(4):
                        fq = []
                        if pair < 3:
                            for c in range(NCX):
                                fq += proj_closures(pair + 1, c)
                                fq += rope_closures(pair + 1, c, False)
                                fq += rope_closures(pair + 1, c, True)
                        state = {"points": 2 * NA * NIC}

                        def drain():
                            k = -(-len(fq) // max(1, state["points"]))
                            state["points"] -= 1
                            while k > 0 and fq:
                                fq.pop(0)()
                                k -= 1

                        for ic in range(NIC):
                            ps_o = [ps.tile([P, IC], F32, tag="ps_o0", bufs=1,
                                            name=f"ps_o0_{pair}_{ic}"),
                                    ps.tile([P, IC], F32, tag="ps_o1", bufs=1,
                                            name=f"ps_o1_{pair}_{ic}")]
                            for a in range(NA):
                                pts = {}
                                scores_exp(pair, ic, a, 0, pts)
                                scores_exp(pair, ic, a, 1, pts)
                                drain()
                                pv(pair, ic, a, 0, ps_o, pts)
                                pv(pair, ic, a, 1, ps_o, pts)
                                drain()
                                if pair == 0 and ic == 0 and a + 2 < NA:
                                    emit_v(a + 2)
                            norm(pair, ic, ps_o)
                            if pair == 3:
                                for rt in range(4 * ic, 4 * ic + 4):
                                    fq += out_closures(rt)
                        for f in fq:
                            f()

    if split_waits:
        _split_sync_waits(nc)
    return nc


_SYNC_EXEMPT = {"InstEventSemaphore", "InstAllEngineBarrier",
                "InstNoOp", "InstUnconditionalBranch", "InstCompareAndBranch",
                "InstHalt", "InstBranchHint"}


def _split_sync_waits(nc, cap_total=2):
    """Compact TPB instruction encodings only fit ~2 sync commands
    (waits+updates).  Tile can park many waits on one instruction; hoist the
    excess onto same-engine InstNoOps inserted immediately before (waits
    strictly earlier in the same stream — always safe)."""
    for fn in nc.m.functions:
        for bb in fn.blocks:
            il = bb.instructions
            n = 0
            while n < len(il):
                i = il[n]
                nm = type(i).__name__
                si = i.sync_info
                if nm in _SYNC_EXEMPT or si is None:
                    n += 1
                    continue
                waits = list(si.on_wait or [])
                upds = list(si.on_update or [])
                allowed = 0 if len(upds) >= 2 else 1
                if len(waits) <= allowed:
                    n += 1
                    continue
                keep = waits[-allowed:] if allowed else []
                excess = waits[:len(waits) - allowed]
                pos = n
                while excess:
                    chunk, excess = excess[:1], excess[1:]
                    nop = mybir.InstNoOp(
                        name=nc.get_next_instruction_name(),
                        engine=i.engine,
                        bass_nofuse=True,
                        sync_info=mybir.SyncInfo(on_wait=chunk, on_update=[]),
                    )
                    il.insert(pos, nop)
                    pos += 1
                si.on_wait = keep
                n = pos + 1


def _rot_matrix_T():
    """R^T such that (R @ tT) = rotate_half(t)^T in [h*64+d, n] layout."""
    r64 = np.zeros((DH, DH), dtype=np.float32)
    for dp in range(32):
        r64[dp, dp + 32] = -1.0
        r64[dp + 32, dp] = 1.0
    r = np.zeros((P, P), dtype=np.float32)
    r[:DH, :DH] = r64
    r[DH:, DH:] = r64
    return np.ascontiguousarray(r.T.astype(NP_BF16))


def make_in_maps(x, rotary_emb, Wq, Wkv, Wout, n_cores=8):
    B, N, Dm = x.shape
    rT = _rot_matrix_T()
    cosT = np.tile(np.cos(rotary_emb.astype(np.float64)).T, (2, 1)).astype(NP_BF16)
    sinT = np.tile(np.sin(rotary_emb.astype(np.float64)).T, (2, 1)).astype(NP_BF16)
    cosT = np.ascontiguousarray(cosT)
    sinT = np.ascontiguousarray(sinT)
    wk_full = Wkv[:, :Dm]
    wv_full = Wkv[:, Dm:]
    in_maps = []
    for c in range(n_cores):
        b, hh = c // 2, c % 2
        sl = slice(hh * INNER, (hh + 1) * INNER)
        in_maps.append({
            "xT": np.ascontiguousarray(x[b].T).astype(NP_BF16),
            "wq": np.ascontiguousarray(Wq[:, sl]).astype(NP_BF16),
            "wk": np.ascontiguousarray(wk_full[:, sl]).astype(NP_BF16),
            "wv": np.ascontiguousarray(wv_full[:, sl]).astype(NP_BF16),
            "wout": np.ascontiguousarray(Wout[sl, :]).astype(NP_BF16),
            "cosT": cosT,
            "sinT": sinT,
            "rT": rT,
        })
    return in_maps


_NC_CACHE = {}


def kernel(x, rotary_emb, Wq, Wkv, Wout, bout, _trace=False):
    B, N, Dm = x.shape
    if "nc" not in _NC_CACHE:
        _NC_CACHE["nc"] = build_nc(N=N)
    nc = _NC_CACHE["nc"]
    in_maps = make_in_maps(x, rotary_emb, Wq, Wkv, Wout)
    res = run_bass_kernel_spmd(nc, in_maps, core_ids=list(range(8)),
                               trace=_trace)
    outs = [res.results[c]["out"] for c in range(8)]
    full = np.empty((B, N, Dm), dtype=np.float32)
    for b in range(B):
        full[b] = outs[2 * b] + outs[2 * b + 1] + bout[None, :].astype(np.float32)
    if _trace:
        return full, res
    return full


# revision 19
# speedup vs baseline: 2.1763x; 1.0872x over previous
"""Trainium2 Bass kernel for nn_Attention (B=4, N=2048, D=1024, H=16, DH=64).

Sharding: 8 cores = 4 batches x 2 head-halves (tensor-parallel heads).
Per core: qT/kT/v projections (bf16 matmuls, fp32 psum) -> RoPE (PE
permutation matmul + DVE combine, written directly to qTr/kTr) -> scores^T
per jt-pair into a [128,2,512] psum -> ONE 1024-wide exp per (head,
jt-pair) -> P^T @ [v|1] per jt (fused row-sum via ones column) -> deferred
normalization (reciprocal + fp32 DMA partition-broadcast + DVE multiply)
-> output projection partial (Wout row-shard).
Host: out[b] = partial[2b] + partial[2b+1] + bout.

Scheduling: the PE executes in emission order, so the attention inner loop
(ACT-bound: 2.08us exp vs 1.71us matmul per group) is interleaved with
micro-fillers -- next pair's projection/rope matmuls, v-projection (pair 0),
and output-projection chunks (pair 3) -- emitted at points inside each
group where PE would otherwise stall on the exp->PV dependency.

Engine budget per core per pass: PE ~765k matmul-columns (qkv 196k, S^T
262k, PV 262k, outproj 65k, rope 16k); ACT 256 1024-wide exps; DVE ~600
ops.  Measured steady-state marginal on hw: ~165us/pass quiet, ~240-340us
under load (device throttles on sustained runs); 1.74x the 414us baseline
under matched conditions.
"""

import sys

for _p in ("/opt/trn_rl_repo",):
    if _p not in sys.path:
        sys.path.insert(0, _p)

import numpy as np
import ml_dtypes

import concourse.bass as bass
import concourse.tile as tile
from concourse import library_config, mybir
from concourse.bass_utils import run_bass_kernel_spmd

BF16 = mybir.dt.bfloat16
F32 = mybir.dt.float32
NP_BF16 = ml_dtypes.bfloat16
EXP = mybir.ActivationFunctionType.Exp
IDENT = mybir.ActivationFunctionType.Identity

P = 128          # partitions
D = 1024         # model dim
INNER = 512      # per-core inner dim (8 heads * 64)
NH = 8           # heads per core
DH = 64          # head dim
KT = D // P      # 8 contraction tiles for projections
SCALE = DH ** -0.5


def build_nc(N=2048, n_cores=8, split_waits=True, repeat=1):
    """Build the per-core bass program (SPMD: same program, different data)."""
    NJT = N // P           # 16 j tiles
    NA = NJT // 2          # 8 jt-pairs
    IC = 512               # i-chunk
    NIC = N // IC          # 4
    PCH = 512              # projection col chunk
    NCX = N // PCH         # 4

    nc = bass.Bass("TRN2", target_bir_lowering=False, debug=False,
                   num_devices=n_cores)

    xT = nc.dram_tensor("xT", [D, N], BF16, kind="ExternalInput").ap()
    wq = nc.dram_tensor("wq", [D, INNER], BF16, kind="ExternalInput").ap()
    wk = nc.dram_tensor("wk", [D, INNER], BF16, kind="ExternalInput").ap()
    wv = nc.dram_tensor("wv", [D, INNER], BF16, kind="ExternalInput").ap()
    wout = nc.dram_tensor("wout", [INNER, D], BF16, kind="ExternalInput").ap()
    cosT = nc.dram_tensor("cosT", [P, N], BF16, kind="ExternalInput").ap()
    sinT = nc.dram_tensor("sinT", [P, N], BF16, kind="ExternalInput").ap()
    rT = nc.dram_tensor("rT", [P, P], BF16, kind="ExternalInput").ap()
    out = nc.dram_tensor("out", [N, D], BF16, kind="ExternalOutput").ap()

    with tile.TileContext(nc) as tc:
        with tc.tile_pool(name="persist", bufs=1) as pp:
            wq_sb = pp.tile([P, KT, INNER], BF16, tag="wq")
            wk_sb = pp.tile([P, KT, INNER], BF16, tag="wk")
            wv_sb = pp.tile([P, KT, INNER], BF16, tag="wv")
            wout_sb = pp.tile([P, 4, D], BF16, tag="wout")
            cos_sb = pp.tile([P, N], BF16, tag="cos")
            sin_sb = pp.tile([P, N], BF16, tag="sin")
            rT_sb = pp.tile([P, P], BF16, tag="rT")
            xt_all = pp.tile([P, KT, N], BF16, tag="xt")
            qT = pp.tile([P, 4, N], BF16, tag="qT")
            kT = pp.tile([P, 4, N], BF16, tag="kT")
            qTr = pp.tile([P, 4, N], BF16, tag="qTr")
            kTr = pp.tile([P, 4, N], BF16, tag="kTr")
            vA = pp.tile([P, NJT, NH, DH + 1], BF16, tag="vA")
            outT = pp.tile([P, 4, N], BF16, tag="outT")

            # queue order: wq then x chunk 0 first, so the first
            # projection matmul can start ~6us into the call instead of
            # waiting for the whole weight load.
            xT_r0 = xT.rearrange("(kt p) n -> p kt n", p=P)
            for kt in range(KT):
                nc.sync.dma_start(out=wq_sb[:, kt, :],
                                  in_=wq.rearrange("(kt p) i -> p kt i", p=P)[:, kt, :])
            for kt in range(KT):
                nc.sync.dma_start(out=xt_all[:, kt, 0:512],
                                  in_=xT_r0[:, kt, 0:512])
            for kt in range(KT):
                nc.sync.dma_start(out=wk_sb[:, kt, :],
                                  in_=wk.rearrange("(kt p) i -> p kt i", p=P)[:, kt, :])
                nc.sync.dma_start(out=wv_sb[:, kt, :],
                                  in_=wv.rearrange("(kt p) i -> p kt i", p=P)[:, kt, :])
            for p4 in range(4):
                nc.sync.dma_start(out=wout_sb[:, p4, :],
                                  in_=wout.rearrange("(pt p) d -> p pt d", p=P)[:, p4, :])
            nc.sync.dma_start(out=cos_sb, in_=cosT)
            nc.sync.dma_start(out=sin_sb, in_=sinT)
            nc.sync.dma_start(out=rT_sb, in_=rT)
            # ones column for fused row-sum in PV matmul (on ACT so the PV
            # matmul's waits stay within the MM sync-command limit)
            nc.scalar.activation(
                vA[:, :, :, DH:DH + 1],
                cos_sb[:, 0:NJT * NH].rearrange("p (a b) -> p a b", a=NJT)[:, :, :, None],
                IDENT, bias=1.0, scale=0.0)

            for _rep in range(repeat):
                with tc.tile_pool(name="pt_pool", bufs=3) as ptp, \
                     tc.tile_pool(name="nrm", bufs=1) as nrm, \
                     tc.tile_pool(name="rope_t", bufs=2) as rp, \
                     tc.tile_pool(name="fin_p", bufs=2) as fin_p, \
                     tc.tile_pool(name="ps", bufs=1, space="PSUM") as ps:
                    xT_r = xT.rearrange("(kt p) n -> p kt n", p=P)
                    # chunk 0 of rep 0 was already loaded in the persist scope
                    for xc in (range(1, NCX) if _rep == 0 else range(NCX)):
                        xsl = slice(xc * PCH, (xc + 1) * PCH)
                        for kt in range(KT):
                            nc.sync.dma_start(out=xt_all[:, kt, xsl],
                                              in_=xT_r[:, kt, xsl])

                    _ctr = {"n": 0}

                    def aux():
                        _ctr["n"] += 1
                        return ps.tile([P, 2, 512], F32, tag="ps_aux", bufs=1,
                                       name=f"aux{_ctr['n']}")

                    # ---- filler micro-units (closures, ~1 PE matmul each) ----
                    def proj_closures(pair, c):
                        """q+k projection for col chunk c of `pair`."""
                        csl = slice(c * PCH, (c + 1) * PCH)
                        msl = slice(pair * P, (pair + 1) * P)
                        cl = []
                        box = {}

                        def mk(kt, half, w_sb):
                            def f():
                                if "t" not in box:
                                    box["t"] = aux()
                                nc.tensor.matmul(box["t"][:, half, :],
                                                 w_sb[:, kt, msl],
                                                 xt_all[:, kt, csl],
                                                 start=(kt == 0), stop=(kt == KT - 1))
                            return f

                        for kt in range(KT):
                            cl.append(mk(kt, 0, wq_sb))
                        cl.append(lambda: nc.vector.tensor_copy(
                            qT[:, pair, csl], box["t"][:, 0, :]))
                        for kt in range(KT):
                            cl.append(mk(kt, 1, wk_sb))
                        cl.append(lambda: nc.vector.tensor_copy(
                            kT[:, pair, csl], box["t"][:, 1, :]))
                        return cl

                    def rope_closures(pair, c, is_k):
                        csl = slice(c * PCH, (c + 1) * PCH)
                        src, dst = (kT, kTr) if is_k else (qT, qTr)
                        half = 1 if is_k else 0
                        box = {}

                        def rot():
                            box["t"] = aux()
                            nc.tensor.matmul(box["t"][:, half, :], rT_sb,
                                             src[:, pair, csl],
                                             start=True, stop=True)

                        def comb():
                            t1 = rp.tile([P, PCH], BF16, tag="t1")
                            nc.vector.tensor_mul(t1, src[:, pair, csl],
                                                 cos_sb[:, csl])
                            t2 = rp.tile([P, PCH], BF16, tag="t2")
                            nc.vector.tensor_mul(t2, box["t"][:, half, :],
                                                 sin_sb[:, csl])
                            nc.vector.tensor_add(dst[:, pair, csl], t1, t2)

                        return [rot, comb]

                    def emit_v(a):
                        """v projection for j-tiles 2a, 2a+1 (one psum fill)."""
                        pv = aux()
                        for s in range(2):
                            jt = 2 * a + s
                            r2sl = slice(jt * P, (jt + 1) * P)
                            for kt in range(KT):
                                nc.tensor.matmul(pv[:, s, :],
                                                 xt_all[:, kt, r2sl],
                                                 wv_sb[:, kt, :],
                                                 start=(kt == 0), stop=(kt == KT - 1))
                        nc.vector.tensor_copy(
                            vA[:, 2 * a:2 * a + 2, :, 0:DH],
                            pv.rearrange("p s (h d) -> p s h d", h=NH))

                    def out_closures(rtile):
                        """output projection for row tile rtile (2 col halves,
                        one psum bank each)."""
                        rsl = slice(rtile * P, (rtile + 1) * P)
                        cl = []
                        box = {}

                        def mk(p4, half):
                            def f():
                                if "t" not in box:
                                    box["t"] = aux()
                                nsl = slice(half * 512, (half + 1) * 512)
                                nc.tensor.matmul(
                                    box["t"][:, half, :],
                                    outT[:, p4, rsl], wout_sb[:, p4, nsl],
                                    start=(p4 == 0), stop=(p4 == 3))
                            return f

                        for half in range(2):
                            for p4 in range(4):
                                cl.append(mk(p4, half))

                        def fin():
                            ft = fin_p.tile([P, D], BF16, tag="fin",
                                            name=f"fin{rtile}")
                            nc.vector.tensor_copy(
                                ft, box["t"].rearrange("p a b -> p (a b)"))
                            nc.sync.dma_start(out=out[rsl, :], in_=ft)
                        cl.append(fin)
                        return cl

                    # ---- attention pieces ----
                    def scores_exp(pair, ic, a, lc, pts):
                        isl = slice(ic * IC, (ic + 1) * IC)
                        prow = slice(lc * DH, (lc + 1) * DH)
                        pssh = ps.tile([P, 2, IC], F32, tag=f"ps_s{lc}", bufs=1,
                                       name=f"pssh{pair}_{ic}_{a}_{lc}")
                        for s in range(2):
                            jt = 2 * a + s
                            jsl = slice(jt * P, (jt + 1) * P)
                            nc.tensor.matmul(pssh[:, s, :],
                                             kTr[prow, pair, jsl],
                                             qTr[prow, pair, isl],
                                             start=True, stop=True)
                        pt = ptp.tile([P, 2, IC], BF16, tag=f"pt{lc}",
                                      name=f"pt{pair}_{ic}_{a}_{lc}")
                        nc.scalar.activation(pt, pssh, EXP, scale=SCALE)
                        pts[lc] = pt

                    def pv(pair, ic, a, lc, ps_o, pts):
                        h = pair * 2 + lc
                        for s in range(2):
                            nc.tensor.matmul(ps_o[lc][0:DH + 1, :],
                                             vA[:, 2 * a + s, h, :],
                                             pts[lc][:, s, :],
                                             start=(a == 0 and s == 0),
                                             stop=(a == NA - 1 and s == 1))

                    def norm(pair, ic, ps_o):
                        isl = slice(ic * IC, (ic + 1) * IC)
                        for lc in range(2):
                            prow = slice(lc * DH, (lc + 1) * DH)
                            oc = nrm.tile([DH + 1, IC], F32, tag=f"oc{lc}",
                                          name=f"oc{pair}_{ic}_{lc}")
                            nc.vector.tensor_copy(oc, ps_o[lc][0:DH + 1, :])
                            lrec = nrm.tile([1, IC], F32, tag=f"lrec{lc}",
                                            name=f"lrec{pair}_{ic}_{lc}")
                            nc.vector.reciprocal(lrec, oc[DH:DH + 1, :])
                            lb = nrm.tile([DH, IC], F32, tag=f"lb{lc}",
                                          name=f"lb{pair}_{ic}_{lc}")
                            nc.sync.dma_start(
                                out=lb,
                                in_=lrec[:, None, :].to_broadcast((1, DH, IC)))
                            nc.vector.tensor_mul(outT[prow, pair, isl],
                                                 oc[0:DH, :], lb)

                    # ---- warmup: pair-0 proj+rope, first v pairs ----
                    for c in range(NCX):
                        for f in proj_closures(0, c):
                            f()
                    for c in range(NCX):
                        for f in rope_closures(0, c, False):
                            f()
                        for f in rope_closures(0, c, True):
                            f()
                    emit_v(0)
                    emit_v(1)

                    # ---- main: attention with interleaved fillers ----
                    for pair in range(4):
                        fq = []
                        if pair < 3:
                            for c in range(NCX):
                                fq += proj_closures(pair + 1, c)
                                fq += rope_closures(pair + 1, c, False)
                                fq += rope_closures(pair + 1, c, True)
                        state = {"points": 2 * NA * NIC}

                        def drain():
                            k = -(-len(fq) // max(1, state["points"]))
                            state["points"] -= 1
                            while k > 0 and fq:
                                fq.pop(0)()
                                k -= 1

                        for ic in range(NIC):
                            ps_o = [ps.tile([P, IC], F32, tag="ps_o0", bufs=1,
                                            name=f"ps_o0_{pair}_{ic}"),
                                    ps.tile([P, IC], F32, tag="ps_o1", bufs=1,
                                            name=f"ps_o1_{pair}_{ic}")]
                            for a in range(NA):
                                pts = {}
                                scores_exp(pair, ic, a, 0, pts)
                                scores_exp(pair, ic, a, 1, pts)
                                drain()
                                pv(pair, ic, a, 0, ps_o, pts)
                                pv(pair, ic, a, 1, ps_o, pts)
                                drain()
                                if pair == 0 and ic == 0 and a + 2 < NA:
                                    emit_v(a + 2)
                            norm(pair, ic, ps_o)
                            if pair == 3:
                                for rt in range(4 * ic, 4 * ic + 4):
                                    fq += out_closures(rt)
                        for f in fq:
                            f()

    if split_waits:
        _split_sync_waits(nc)
    return nc


_SYNC_EXEMPT = {"InstEventSemaphore", "InstAllEngineBarrier",
                "InstNoOp", "InstUnconditionalBranch", "InstCompareAndBranch",
                "InstHalt", "InstBranchHint"}


def _split_sync_waits(nc, cap_total=2):
    """Compact TPB instruction encodings only fit ~2 sync commands
    (waits+updates).  Tile can park many waits on one instruction; hoist the
    excess onto same-engine InstNoOps inserted immediately before (waits
    strictly earlier in the same stream — always safe)."""
    for fn in nc.m.functions:
        for bb in fn.blocks:
            il = bb.instructions
            n = 0
            while n < len(il):
                i = il[n]
                nm = type(i).__name__
                si = i.sync_info
                if nm in _SYNC_EXEMPT or si is None:
                    n += 1
                    continue
                waits = list(si.on_wait or [])
                upds = list(si.on_update or [])
                allowed = 0 if len(upds) >= 2 else 1
                if len(waits) <= allowed:
                    n += 1
                    continue
                keep = waits[-allowed:] if allowed else []
                excess = waits[:len(waits) - allowed]
                pos = n
                while excess:
                    chunk, excess = excess[:1], excess[1:]
                    nop = mybir.InstNoOp(
                        name=nc.get_next_instruction_name(),
                        engine=i.engine,
                        bass_nofuse=True,
                        sync_info=mybir.SyncInfo(on_wait=chunk, on_update=[]),
                    )
                    il.insert(pos, nop)
                    pos += 1
                si.on_wait = keep
                n = pos + 1


def _rot_matrix_T():
    """R^T such that (R @ tT) = rotate_half(t)^T in [h*64+d, n] layout."""
    r64 = np.zeros((DH, DH), dtype=np.float32)
    for dp in range(32):
        r64[dp, dp + 32] = -1.0
        r64[dp + 32, dp] = 1.0
    r = np.zeros((P, P), dtype=np.float32)
    r[:DH, :DH] = r64
    r[DH:, DH:] = r64
    return np.ascontiguousarray(r.T.astype(NP_BF16))


def make_in_maps(x, rotary_emb, Wq, Wkv, Wout, n_cores=8):
    B, N, Dm = x.shape
    rT = _rot_matrix_T()
    cosT = np.tile(np.cos(rotary_emb.astype(np.float64)).T, (2, 1)).astype(NP_BF16)
    sinT = np.tile(np.sin(rotary_emb.astype(np.float64)).T, (2, 1)).astype(NP_BF16)
    cosT = np.ascontiguousarray(cosT)
    sinT = np.ascontiguousarray(sinT)
    wk_full = Wkv[:, :Dm]
    wv_full = Wkv[:, Dm:]
    in_maps = []
    for c in range(n_cores):
        b, hh = c // 2, c % 2
        sl = slice(hh * INNER, (hh + 1) * INNER)
        in_maps.append({
            "xT": np.ascontiguousarray(x[b].T).astype(NP_BF16),
            "wq": np.ascontiguousarray(Wq[:, sl]).astype(NP_BF16),
            "wk": np.ascontiguousarray(wk_full[:, sl]).astype(NP_BF16),
            "wv": np.ascontiguousarray(wv_full[:, sl]).astype(NP_BF16),
            "wout": np.ascontiguousarray(Wout[sl, :]).astype(NP_BF16),
            "cosT": cosT,
            "sinT": sinT,
            "rT": rT,
        })
    return in_maps


_NC_CACHE = {}


def kernel(x, rotary_emb, Wq, Wkv, Wout, bout, _trace=False):
    B, N, Dm = x.shape
    if "nc" not in _NC_CACHE:
        _NC_CACHE["nc"] = build_nc(N=N)
    nc = _NC_CACHE["nc"]
    in_maps = make_in_maps(x, rotary_emb, Wq, Wkv, Wout)
    res = run_bass_kernel_spmd(nc, in_maps, core_ids=list(range(8)),
                               trace=_trace)
    outs = [res.results[c]["out"].astype(np.float32) for c in range(8)]
    full = np.empty((B, N, Dm), dtype=np.float32)
    for b in range(B):
        full[b] = outs[2 * b] + outs[2 * b + 1] + bout[None, :].astype(np.float32)
    if _trace:
        return full, res
    return full
